# revision 1
# baseline (speedup 1.0000x reference)
"""Cross-cryptocurrency attention kernel for 8 Trainium2 NeuronCores.

Sharding: 16 (batch, seq-quarter) shards -> core c handles b = c//4,
query rows s in [512*(c%4), 512*(c%4+1)).  Each core computes all 8 heads
and all 9 (query-asset, key-asset) pairs for its query slice, with full
keys/values (S=2048) for its batch, so the output projection is local and
no collectives are needed.

Pipeline per (head, i, j):
  scores^T[t,s] on PE (K=hd=32, lhsT=k^T, rhs=q^T, bf16)
  exp on ACT (PSUM->SBUF bf16, scale=1/sqrt(hd))  <- bottleneck engine
  AV on PE: lhsT=[v|1] (ones column yields row-sums), rhs=E, accumulate PSUM
  normalize: PE-transpose O^T -> natural, DVE reciprocal + per-partition mul
"""

import math
import numpy as np

B = 2
S = 2048
D = 256
H = 8
HD = 32
SQ = 512  # query rows per core
N_CORES = 8
SCALE = 1.0 / math.sqrt(HD)

_CACHE = {}


def _build():
    from contextlib import ExitStack

    import concourse.bass as bass
    import concourse.mybir as mybir
    import concourse.tile as tile
    from concourse import bacc
    from concourse.masks import make_identity

    f32 = mybir.dt.float32
    bf16 = mybir.dt.bfloat16
    AF = mybir.ActivationFunctionType

    nc = bacc.Bacc("TRN2", target_bir_lowering=False, debug=False)

    x_d = nc.dram_tensor("x", [3, S, D], f32, kind="ExternalInput").ap()
    Wq_d = nc.dram_tensor("Wq", [3, D, D], f32, kind="ExternalInput").ap()
    bq_d = nc.dram_tensor("bq", [3, D], f32, kind="ExternalInput").ap()
    Wk_d = nc.dram_tensor("Wk", [3, D, D], f32, kind="ExternalInput").ap()
    bk_d = nc.dram_tensor("bk", [3, D], f32, kind="ExternalInput").ap()
    Wv_d = nc.dram_tensor("Wv", [3, D, D], f32, kind="ExternalInput").ap()
    bv_d = nc.dram_tensor("bv", [3, D], f32, kind="ExternalInput").ap()
    Wo_d = nc.dram_tensor("Wo", [D, D], f32, kind="ExternalInput").ap()
    bo_d = nc.dram_tensor("bo", [D], f32, kind="ExternalInput").ap()
    out_d = nc.dram_tensor("out", [3, SQ, D], f32, kind="ExternalOutput").ap()

    with tile.TileContext(nc) as tc, ExitStack() as ctx:
        # ---- persistent SBUF pools (bufs=1 == plain buffers) ----
        const_p = ctx.enter_context(tc.tile_pool(name="const", bufs=1))
        xT_p = ctx.enter_context(tc.tile_pool(name="xT", bufs=1))
        qkv_p = ctx.enter_context(tc.tile_pool(name="qkv", bufs=1))
        acc_p = ctx.enter_context(tc.tile_pool(name="acc", bufs=1))
        # streaming pools
        xn_p = ctx.enter_context(tc.tile_pool(name="xn", bufs=2))
        e_p = ctx.enter_context(tc.tile_pool(name="epool", bufs=4))
        sm_p = ctx.enter_context(tc.tile_pool(name="small", bufs=2))
        # PSUM: 6 + 1 + 1 = 8 banks
        ps_S = ctx.enter_context(tc.tile_pool(name="psS", bufs=2, space="PSUM"))
        ps_O = ctx.enter_context(tc.tile_pool(name="psO", bufs=1, space="PSUM"))
        ps_N = ctx.enter_context(tc.tile_pool(name="psN", bufs=1, space="PSUM"))

        # ---- constants / weights to SBUF ----
        ident = const_p.tile([128, 128], f32)
        make_identity(nc, ident[:])
        ones = const_p.tile([1, 128], f32)
        nc.gpsimd.memset(ones[:], 1.0)

        wq_sb = const_p.tile([128, 3 * 2 * D], f32)
        wk_sb = const_p.tile([128, 3 * 2 * D], f32)
        wv_sb = const_p.tile([128, 3 * 2 * D], f32)
        for w_sb, w_d in ((wq_sb, Wq_d), (wk_sb, Wk_d), (wv_sb, Wv_d)):
            nc.sync.dma_start(
                w_sb[:].rearrange("p (a kt f) -> p a kt f", a=3, kt=2),
                w_d.rearrange("a (kt p) f -> p a kt f", p=128),
            )
        wo_sb = const_p.tile([128, 2 * D], f32)
        nc.sync.dma_start(
            wo_sb[:].rearrange("p (kt f) -> p kt f", kt=2),
            Wo_d.rearrange("(kt p) f -> p kt f", p=128),
        )
        bqk_sb = const_p.tile([128, 12], f32)  # col = ty*6 + a*2 + dt (ty: q=0,k=1)
        nc.sync.dma_start(
            bqk_sb[:, 0:6].rearrange("p (a dt) -> p a dt", a=3),
            bq_d.rearrange("a (dt p) -> p a dt", p=128),
        )
        nc.sync.dma_start(
            bqk_sb[:, 6:12].rearrange("p (a dt) -> p a dt", a=3),
            bk_d.rearrange("a (dt p) -> p a dt", p=128),
        )
        bv_row = const_p.tile([1, 3 * D], f32)
        nc.sync.dma_start(bv_row[:], bv_d.rearrange("a f -> (a f)")[None, :])
        bo_row = const_p.tile([1, D], f32)
        nc.sync.dma_start(bo_row[:], bo_d[None, :])

        # ---- per-asset persistent tensors ----
        xT = [xT_p.tile([128, 2 * S], f32, tag=f"xT{_}", name=f"xT{_}") for _ in range(3)]
        kT = [qkv_p.tile([128, 2 * S], bf16, tag=f"kT{_}", name=f"kT{_}") for _ in range(3)]
        qT = [qkv_p.tile([128, 2 * SQ], bf16, tag=f"qT{_}", name=f"qT{_}") for _ in range(3)]
        v1 = [qkv_p.tile([128, 16 * (H * 33)], bf16, tag=f"v1_{_}", name=f"v1_{_}") for _ in range(3)]
        out_acc = [acc_p.tile([128, 4 * D], f32, tag=f"oacc{_}", name=f"oacc{_}") for _ in range(3)]

        # ======== Phase 1: load x, transpose, project q/k/v ========
        for a in range(3):
            xn = xn_p.tile([128, 16 * D], f32)
            for c in range(4):
                nc.sync.dma_start(
                    xn[:, c * 4 * D : (c + 1) * 4 * D].rearrange(
                        "p (st d) -> p st d", st=4
                    ),
                    x_d[a].rearrange("(st p) d -> p st d", p=128)[:, 4 * c : 4 * c + 4],
                )
            # transpose x -> xT  (16 s-tiles x 2 d-tiles)
            for dt in range(2):
                for g in range(4):  # groups of 4 s-tiles per PSUM bank
                    pst = ps_S.tile([128, 512], f32, tag="psS", name="ps1")
                    for u in range(4):
                        st = 4 * g + u
                        nc.tensor.matmul(
                            pst[:, u * 128 : (u + 1) * 128],
                            xn[:, st * D + dt * 128 : st * D + dt * 128 + 128],
                            ident[:],
                            start=True,
                            stop=True,
                        )
                    nc.vector.tensor_copy(
                        xT[a][:, dt * S + g * 512 : dt * S + (g + 1) * 512], pst[:]
                    )
            # k^T projection: psum[dout(128), t(512)] ; +bias ; -> bf16
            for dt in range(2):
                for tc4 in range(4):
                    psk = ps_S.tile([128, 512], f32, tag="psS", name="ps1")
                    for kt in range(2):
                        nc.tensor.matmul(
                            psk[:],
                            wk_sb[:, a * 2 * D + kt * D + dt * 128 : a * 2 * D + kt * D + dt * 128 + 128],
                            xT[a][:, kt * S + tc4 * 512 : kt * S + (tc4 + 1) * 512],
                            start=(kt == 0),
                            stop=(kt == 1),
                        )
                    nc.vector.tensor_scalar_add(
                        kT[a][:, dt * S + tc4 * 512 : dt * S + (tc4 + 1) * 512],
                        psk[:],
                        bqk_sb[:, 6 + a * 2 + dt : 7 + a * 2 + dt],
                    )
            # q^T projection for this core's query slice (dynamic col offset)
            for dt in range(2):
                psq = ps_S.tile([128, 512], f32, tag="psS", name="ps1")
                for kt in range(2):
                    nc.tensor.matmul(
                        psq[:],
                        wq_sb[:, a * 2 * D + kt * D + dt * 128 : a * 2 * D + kt * D + dt * 128 + 128],
                        xT[a][:, kt * S : kt * S + SQ],
                        start=(kt == 0),
                        stop=(kt == 1),
                    )
                nc.vector.tensor_scalar_add(
                    qT[a][:, dt * SQ : (dt + 1) * SQ],
                    psq[:],
                    bqk_sb[:, a * 2 + dt : 1 + a * 2 + dt],
                )
            # v projection: psum[s(128), dout(256)] ; bias via ones-row matmul
            nc.gpsimd.memset(
                v1[a].rearrange("p (t h x) -> p (t h) x", h=H, x=33)[:, :, 32:33],
                1.0,
            )
            for st in range(16):
                psv = ps_O.tile([128, D], f32, tag="psO", name="psv")
                for kt in range(2):
                    nc.tensor.matmul(
                        psv[:],
                        xT[a][:, kt * S + st * 128 : kt * S + (st + 1) * 128],
                        wv_sb[:, a * 2 * D + kt * D : a * 2 * D + (kt + 1) * D],
                        start=(kt == 0),
                        stop=False,
                    )
                nc.tensor.matmul(
                    psv[:],
                    ones[0:1, 0:128],
                    bv_row[0:1, a * D : (a + 1) * D],
                    start=False,
                    stop=True,
                )
                dst = v1[a][
                    :, st * (H * 33) : (st + 1) * (H * 33)
                ].rearrange("p (h x) -> p h x", x=33)[:, :, 0:32]
                nc.vector.tensor_copy(
                    dst, psv[:].rearrange("p (h x) -> p h x", x=32)
                )

        # ======== Phase 2: attention ========
        GROUPS = [(0, 3), (3, 3), (6, 3), (9, 3), (12, 3), (15, 1)]
        for h in range(H):
            hp = 32 * (h % 4)  # partition base for this head
            hc = h // 4  # d-tile containing this head
            for i in range(3):
                for j in range(3):
                    psO = ps_O.tile([33, 512], f32, tag="psO", name="psO")
                    for t0, glen in GROUPS:
                        psS = ps_S.tile([128, glen * 512], f32, tag="psS")
                        for u in range(glen):
                            tt = t0 + u
                            nc.tensor.matmul(
                                psS[:, u * 512 : (u + 1) * 512],
                                kT[j][hp : hp + 32, hc * S + tt * 128 : hc * S + (tt + 1) * 128],
                                qT[i][hp : hp + 32, hc * SQ : (hc + 1) * SQ],
                                start=True,
                                stop=True,
                                tile_position=(hp, 0),
                            )
                        eg = e_p.tile([128, 3 * 512], bf16, tag="eg")
                        nc.scalar.activation(
                            eg[:, 0 : glen * 512], psS[:], AF.Exp, scale=SCALE
                        )
                        for u in range(glen):
                            tt = t0 + u
                            nc.tensor.matmul(
                                psO[:],
                                v1[j][:, tt * (H * 33) + h * 33 : tt * (H * 33) + (h + 1) * 33],
                                eg[:, u * 512 : (u + 1) * 512],
                                start=(tt == 0),
                                stop=(tt == 15),
                            )
                    # normalize: copy to SBUF, PE-transpose to natural [s, d+1]
                    osb = sm_p.tile([33, 512], f32, tag="osb")
                    nc.vector.tensor_copy(osb[:], psO[:])
                    psN = ps_N.tile([128, 4 * 34], f32, tag="psN")
                    for k in range(4):
                        nc.tensor.matmul(
                            psN[:, k * 34 : k * 34 + 33],
                            osb[0:33, k * 128 : (k + 1) * 128],
                            ident[0:33, 0:33],
                            start=True,
                            stop=True,
                        )
                    rr = sm_p.tile([128, 4], f32, tag="rr")
                    nc.vector.reciprocal_approx_fast(
                        rr[:],
                        psN[:].rearrange("p (k x) -> p k x", x=34)[:, :, 32],
                    )
                    oa_view = out_acc[i].rearrange("p (k d) -> p k d", d=D)[
                        :, :, h * 32 : (h + 1) * 32
                    ]
                    if j == 0:
                        for k in range(4):
                            nc.vector.tensor_scalar_mul(
                                oa_view[:, k, :],
                                psN[:, k * 34 : k * 34 + 32],
                                rr[:, k : k + 1],
                            )
                    else:
                        tmp = sm_p.tile([128, 4 * 32], f32, tag="tmp")
                        for k in range(4):
                            nc.vector.tensor_scalar_mul(
                                tmp[:, k * 32 : (k + 1) * 32],
                                psN[:, k * 34 : k * 34 + 32],
                                rr[:, k : k + 1],
                            )
                        nc.vector.tensor_add(
                            oa_view,
                            oa_view,
                            tmp[:].rearrange("p (k d) -> p k d", d=32),
                        )

        # ======== Phase 3: output projection ========
        for a in range(3):
            aT = acc_p.tile([128, 2 * SQ], f32, tag="aT")
            for dt in range(2):
                pst = ps_S.tile([128, 512], f32, tag="psS", name="ps3")
                for st in range(4):
                    nc.tensor.matmul(
                        pst[:, st * 128 : (st + 1) * 128],
                        out_acc[a][:, st * D + dt * 128 : st * D + dt * 128 + 128],
                        ident[:],
                        start=True,
                        stop=True,
                    )
                nc.vector.tensor_copy(aT[:, dt * SQ : (dt + 1) * SQ], pst[:])
            for st in range(4):
                psf = ps_O.tile([128, D], f32, tag="psO", name="psf")
                for dt in range(2):
                    nc.tensor.matmul(
                        psf[:],
                        aT[:, dt * SQ + st * 128 : dt * SQ + (st + 1) * 128],
                        wo_sb[:, dt * D : (dt + 1) * D],
                        start=(dt == 0),
                        stop=False,
                    )
                nc.tensor.matmul(
                    psf[:],
                    ones[0:1, 0:128],
                    bo_row[0:1, :],
                    start=False,
                    stop=True,
                )
                ot = sm_p.tile([128, D], f32, tag="ot")
                nc.vector.tensor_copy(ot[:], psf[:])
                nc.sync.dma_start(
                    out_d[a].rearrange("(st p) d -> st p d", p=128)[st], ot[:]
                )

    nc.compile()
    return nc


def kernel(x_btc, x_eth, x_sol, Wq, bq, Wk, bk, Wv, bv, Wo, bo):
    from concourse.bass_utils import run_bass_kernel_spmd

    if "nc" not in _CACHE:
        _CACHE["nc"] = _build()
    nc = _CACHE["nc"]

    xs = [np.ascontiguousarray(np.asarray(t, dtype=np.float32)) for t in (x_btc, x_eth, x_sol)]
    common = {
        "Wq": np.asarray(Wq, np.float32), "bq": np.asarray(bq, np.float32),
        "Wk": np.asarray(Wk, np.float32), "bk": np.asarray(bk, np.float32),
        "Wv": np.asarray(Wv, np.float32), "bv": np.asarray(bv, np.float32),
        "Wo": np.asarray(Wo, np.float32), "bo": np.asarray(bo, np.float32),
    }
    in_maps = []
    for c in range(N_CORES):
        b, sq = c // 4, c % 4
        # Roll the sequence so this core's query quarter sits at rows [0:512)
        # (the kernel always projects q from rows 0:512).  k/v see the rolled
        # full sequence, which is fine: softmax+sum over the key axis is
        # permutation-invariant.
        xq = np.stack(
            [np.roll(xs[i][b], -sq * SQ, axis=0) for i in range(3)]
        ).astype(np.float32)
        in_maps.append({"x": np.ascontiguousarray(xq), **common})
    import os
    res = run_bass_kernel_spmd(
        nc, in_maps, core_ids=list(range(N_CORES)),
        trace=bool(os.environ.get("BASS_TRACE")),
    )
    _CACHE["last_res"] = res

    outs = [np.empty((B, S, D), np.float32) for _ in range(3)]
    for c in range(N_CORES):
        b, sq = c // 4, c % 4
        o = res.results[c]["out"]
        for i in range(3):
            outs[i][b, sq * SQ : (sq + 1) * SQ] = o[i]
    return tuple(outs)


if __name__ == "__main__":
    import reference

    inp = reference.setup_inputs()
    inp = {k: np.asarray(v) for k, v in inp.items()}
    got = kernel(**inp)
    exp = reference.reference(**inp)
    for i in range(3):
        g, e = np.asarray(got[i]), np.asarray(exp[i])
        err = np.abs(g - e).max() / np.abs(e).max()
        print(f"out[{i}] rel err {err:.3e}")

